# Initial kernel scaffold
#
"""Sequence-parallel attention kernel for 8 TRN2 NeuronCores (bf16).

Reference computation (all fp32):
    Q = x @ Wq.T ; K = x @ Wk.T ; V = x @ Wv.T
    S = Q @ K.T / sqrt(1024)
    out = softmax(S, axis=-1) @ V

Math restructure (identical result, minimal device FLOPs):
    At = Wq.T @ Wk                       (host weight folding, [c, b])
    Pt[b, q]  = sum_c At[c, b] xt[c, q]  [1024, 512] per-core
    St[k, q]  = sum_b xt[b, k] Pt[b, q]  (scores transposed, streamed)
    E         = exp(St / 32)             (no max-subtract: |St/32| < ~4)
    esum[p,q] = sum_kb E[kb][p, q]       (DVE adds; host finishes denom)
    Ut[c, q]  = sum_k x[k, c] E[k, q]    (PSUM-accumulated chains)
    out[q,dv] = sum_c Ut[c, q] WvT[c, dv]   (unnormalized, bf16)
    host: out / denom[q]

Each core handles 512 query rows against the full key range: slightly
UNDER 1/8 of the reference FLOP count (the host weight folding removes
the G = Wq @ xblk.T stage), 640 matmuls of [128 x 128 x 512].
All operands are bf16 (full PE rate, half the DMA/SBUF of fp32r, and
fast LDWEIGHTS); PSUM accumulation is fp32 throughout.
"""

import sys

sys.path.insert(0, "/opt/trn_rl_repo")

import ml_dtypes
import numpy as np

import concourse.tile as tile
from concourse import bacc, mybir
from concourse.bass_utils import run_bass_kernel_spmd

F32 = mybir.dt.float32
BF16 = mybir.dt.bfloat16
BF16_NP = ml_dtypes.bfloat16

S = 4096          # sequence length
D = 1024          # d_in == d_out
P = 128           # partitions
NCORES = 8
R = S // NCORES   # query rows per core (512)
NF = 512          # moving free-dim chunk (1 psum bank of fp32)
KSC = 512         # key super-chunk (xt DMA granularity)
NSC = S // KSC    # 8 super-chunks
KB = S // P       # 32 key blocks
DC = D // P       # 8 chunks of the model dim
QC = R // P       # 4 query chunks per core
SCALE = 1.0 / np.sqrt(np.float32(D))

EXP = mybir.ActivationFunctionType.Exp


def build_program():
    nc = bacc.Bacc("TRN2", target_bir_lowering=False, debug=False,
                   num_devices=NCORES)

    x_d = nc.dram_tensor("x", [S, D], BF16, kind="ExternalInput").ap()
    xt_d = nc.dram_tensor("xt", [D, S], BF16, kind="ExternalInput").ap()
    at_d = nc.dram_tensor("at", [D, D], BF16, kind="ExternalInput").ap()
    wvt_d = nc.dram_tensor("wvt", [D, D], BF16, kind="ExternalInput").ap()
    xqt_d = nc.dram_tensor("xqt", [D, R], BF16, kind="ExternalInput").ap()
    out_d = nc.dram_tensor("out", [R, D], BF16, kind="ExternalOutput").ap()
    esum_d = nc.dram_tensor("esum", [P, R], F32, kind="ExternalOutput").ap()

    with tile.TileContext(nc) as tc:
        _emit(tc, x_d, xt_d, at_d, wvt_d, xqt_d, out_d, esum_d)

    nc.compile()
    return nc


def _emit(tc, x_d, xt_d, at_d, wvt_d, xqt_d, out_d, esum_d):
    nc = tc.nc
    from contextlib import ExitStack

    with ExitStack() as ctx:
        ps = ctx.enter_context(tc.tile_pool(name="ps", bufs=8, space="PSUM"))
        early = ctx.enter_context(tc.tile_pool(name="early", bufs=1))
        pt_pool = ctx.enter_context(tc.tile_pool(name="pt", bufs=1))
        es_pool = ctx.enter_context(tc.tile_pool(name="es", bufs=1))
        xnat_pool = ctx.enter_context(tc.tile_pool(name="xnat", bufs=KB))
        xts_pool = ctx.enter_context(tc.tile_pool(name="xts", bufs=24))
        e_pool = ctx.enter_context(tc.tile_pool(name="epool", bufs=KB))
        wvt_pool = ctx.enter_context(tc.tile_pool(name="wvt", bufs=1))
        ut_pool = ctx.enter_context(tc.tile_pool(name="ut", bufs=1))
        out_pool = ctx.enter_context(tc.tile_pool(name="outp", bufs=2))

        # ---- PE warm-up: HAM throttles a cold PE to K=4/8 (~425ns/matmul)
        # and flips to 8/8 only after ~4us of continuous activity.  The PE
        # idles until the first At/xqt chunks land (11.2-13us observed), so
        # run 12 junk matmuls (~4.3us, warm by #10) sized to END inside that
        # arrival window: the real stream then starts at full rate with no
        # idle gap to re-cool the throttle.
        wl = early.tile([P, P], BF16, tag="warml")
        wr = early.tile([P, R], BF16, tag="warmr")
        nc.vector.memset(wl, 0.0)
        nc.vector.memset(wr, 0.0)
        warm_ps = ps.tile([P, R], F32, tag="mm")
        NWARM = 4
        for i in range(NWARM):
            nc.tensor.matmul(warm_ps, wl, wr,
                             start=(i == 0), stop=(i == NWARM - 1))

        xts_sc = {}

        def prefetch_xts(sc):
            tiles = []
            for cb in range(DC):
                t = xts_pool.tile([P, KSC], BF16, tag="xts")
                nc.sync.dma_start(
                    out=t,
                    in_=xt_d[cb * P:(cb + 1) * P, sc * KSC:(sc + 1) * KSC])
                tiles.append(t)
            xts_sc[sc] = tiles

        # ---- prologue DMAs: xqt/At chunk pairs in consumption order.  DMA
        # issue costs ~650ns each on the sync queue, and the chunk pairs
        # pipeline against the ca-outer Pt matmuls below.
        xqt_tiles = []
        at_tiles = []
        for ca in range(DC):
            xq = early.tile([P, R], BF16, tag=f"xqt{ca}")
            nc.sync.dma_start(out=xq, in_=xqt_d[ca * P:(ca + 1) * P, :])
            xqt_tiles.append(xq)
            at = early.tile([P, D], BF16, tag=f"at{ca}")
            if ca == 0:
                # split the stream-gating first chunk across two DMA rings:
                # one ring moves only ~90GB/s, so halves land in ~1.5us
                nc.sync.dma_start(out=at[:, :D // 2],
                                  in_=at_d[:P, :D // 2])
                nc.sync.dma_start(out=at[:, D // 2:],
                                  in_=at_d[:P, D // 2:])
            else:
                nc.sync.dma_start(out=at, in_=at_d[ca * P:(ca + 1) * P, :])
            at_tiles.append(at)
        prefetch_xts(0)
        prefetch_xts(1)

        # ---- Phase Pt: Pt[b, q] = sum_c At[c, b] xqt[c, q] ----
        # ca-outer over 8 parallel PSUM accumulators: matmul ca only needs
        # DMA pair ca, so PE starts as soon as the first pair lands.
        pt_sb = pt_pool.tile([P, DC, R], BF16)
        pt_ps = []
        for _cb in range(DC):
            pt_acc = ps.tile([P, R], F32, tag="mm")
            pt_ps.append(pt_acc)
        for ca in range(DC):
            for cb in range(DC):
                nc.tensor.matmul(
                    pt_ps[cb],
                    at_tiles[ca][:, cb * P:(cb + 1) * P],
                    xqt_tiles[ca],
                    start=(ca == 0), stop=(ca == DC - 1),
                )
        for cb in range(DC):
            if cb % 2 == 0:
                nc.vector.tensor_copy(pt_sb[:, cb, :], pt_ps[cb])
            else:
                nc.scalar.copy(pt_sb[:, cb, :], pt_ps[cb])

        # ---- Phase A: scores streamed over 32 key blocks ----
        esum_sb = es_pool.tile([P, R], F32)
        xnat = []
        e_tiles = []
        for kb in range(KB):
            sc, kin = divmod(kb, KSC // P)
            if kin == 0 and sc + 2 < NSC:
                prefetch_xts(sc + 2)
            # spread the phase-B x loads across phase A, after xts
            xn = xnat_pool.tile([P, D], BF16, tag="xnat")
            nc.sync.dma_start(out=xn, in_=x_d[kb * P:(kb + 1) * P, :])
            xnat.append(xn)

            st_ps = ps.tile([P, R], F32, tag="mm")
            xts = xts_sc[sc]
            for cb in range(DC):
                nc.tensor.matmul(
                    st_ps,
                    xts[cb][:, kin * P:(kin + 1) * P],
                    pt_sb[:, cb, :],
                    start=(cb == 0), stop=(cb == DC - 1),
                )
            et = e_pool.tile([P, R], BF16, tag="e")
            nc.scalar.activation(et, st_ps, EXP, scale=float(SCALE))
            e_tiles.append(et)
            if kb == 0:
                nc.vector.tensor_copy(esum_sb, et)
            else:
                nc.vector.tensor_add(esum_sb, esum_sb, et)
            if sc >= 2 and kin == 0:
                xts_sc.pop(sc - 2, None)

        # wvt is first needed by the out phase; load it during phase B.
        wvt_sb = wvt_pool.tile([P, DC, D], BF16, tag="wvt")
        for cw in range(DC):
            nc.sync.dma_start(out=wvt_sb[:, cw, :],
                              in_=wvt_d[cw * P:(cw + 1) * P, :])
        nc.sync.dma_start(out=esum_d, in_=esum_sb)

        # ---- Phase B: Ut[c, q] accumulated in PSUM over all 32 k-blocks ----
        ut_sb = ut_pool.tile([P, DC, R], BF16)
        for cc in range(DC):
            ut_ps = ps.tile([P, R], F32, tag="mm")
            for kb in range(KB):
                nc.tensor.matmul(
                    ut_ps,
                    xnat[kb][:, cc * P:(cc + 1) * P],
                    e_tiles[kb],
                    start=(kb == 0), stop=(kb == KB - 1),
                )
            if cc % 2 == 0:
                nc.vector.tensor_copy(ut_sb[:, cc, :], ut_ps)
            else:
                nc.scalar.copy(ut_sb[:, cc, :], ut_ps)

        # ---- Phase C: out[q, dv] = sum_c Ut[c, q] WvT[c, dv] (unnormalized;
        # the host divides by the softmax denominator) ----
        for cq in range(QC):
            ot = out_pool.tile([P, D], BF16, tag="out")
            for nd in range(D // NF):
                if cq == QC - 1 and nd == D // NF - 1:
                    # Final group: two 256-wide halves with casts and DMA
                    # issues on parallel engine queues, shortening the
                    # post-last-matmul tail.
                    for h in range(2):
                        lo = nd * NF + h * (NF // 2)
                        ps_h = ps.tile([P, NF // 2], F32, tag="mm")
                        for cc in range(DC):
                            nc.tensor.matmul(
                                ps_h,
                                ut_sb[:, cc, cq * P:(cq + 1) * P],
                                wvt_sb[:, cc, lo:lo + NF // 2],
                                start=(cc == 0), stop=(cc == DC - 1),
                            )
                        if h == 0:
                            nc.vector.tensor_copy(ot[:, lo:lo + NF // 2], ps_h)
                            nc.sync.dma_start(
                                out=out_d[cq * P:(cq + 1) * P, lo:lo + NF // 2],
                                in_=ot[:, lo:lo + NF // 2])
                        else:
                            # last half: one cast, then two quarter DMAs on
                            # separate queues so the final transfers (a
                            # single ring moves only ~90GB/s) run in parallel
                            nc.scalar.copy(ot[:, lo:lo + NF // 2], ps_h)
                            q4 = NF // 4
                            nc.scalar.dma_start(
                                out=out_d[cq * P:(cq + 1) * P, lo:lo + q4],
                                in_=ot[:, lo:lo + q4])
                            nc.gpsimd.dma_start(
                                out=out_d[cq * P:(cq + 1) * P,
                                          lo + q4:lo + 2 * q4],
                                in_=ot[:, lo + q4:lo + 2 * q4])
                    continue
                ps_o = ps.tile([P, NF], F32, tag="mm")
                for cc in range(DC):
                    nc.tensor.matmul(
                        ps_o,
                        ut_sb[:, cc, cq * P:(cq + 1) * P],
                        wvt_sb[:, cc, nd * NF:(nd + 1) * NF],
                        start=(cc == 0), stop=(cc == DC - 1),
                    )
                if nd % 2 == 0:
                    nc.vector.tensor_copy(ot[:, nd * NF:(nd + 1) * NF], ps_o)
                else:
                    nc.scalar.copy(ot[:, nd * NF:(nd + 1) * NF], ps_o)
                # per-half DMA so the final transfer after the last cast is
                # only 128 KiB
                nc.sync.dma_start(
                    out=out_d[cq * P:(cq + 1) * P, nd * NF:(nd + 1) * NF],
                    in_=ot[:, nd * NF:(nd + 1) * NF])


_CACHE = {}


def _get_program():
    if "nc" not in _CACHE:
        _CACHE["nc"] = build_program()
    return _CACHE["nc"]


def make_in_maps(x, W_query, W_key, W_value):
    x32 = np.ascontiguousarray(x, dtype=np.float32)
    xb = x32.astype(BF16_NP)
    xtb = np.ascontiguousarray(x32.T).astype(BF16_NP)
    at = np.ascontiguousarray(
        np.asarray(W_query, dtype=np.float32).T
        @ np.asarray(W_key, dtype=np.float32)).astype(BF16_NP)
    wvt = np.ascontiguousarray(
        np.asarray(W_value, dtype=np.float32).T).astype(BF16_NP)
    maps = []
    for i in range(NCORES):
        xqt = np.ascontiguousarray(xtb[:, i * R:(i + 1) * R])
        maps.append({"x": xb, "xt": xtb, "at": at, "wvt": wvt, "xqt": xqt})
    return maps


def gather_output(results):
    """Normalize per-core outputs and concatenate to the full [S, D] f32."""
    outs = []
    for i in range(NCORES):
        unnorm = np.asarray(results[i]["out"]).astype(np.float32)
        denom = np.asarray(results[i]["esum"]).astype(np.float32).sum(axis=0)
        outs.append(unnorm / denom[:, None])
    return np.concatenate(outs, axis=0)


def kernel(x, W_query, W_key, W_value):
    nc = _get_program()
    in_maps = make_in_maps(x, W_query, W_key, W_value)
    res = run_bass_kernel_spmd(nc, in_maps, core_ids=list(range(NCORES)))
    return gather_output(res.results)



# revision 1
# speedup vs baseline: 1.1004x; 1.1004x over previous
"""Sequence-parallel attention kernel for 8 TRN2 NeuronCores (bf16).

Reference computation (all fp32):
    Q = x @ Wq.T ; K = x @ Wk.T ; V = x @ Wv.T
    S = Q @ K.T / sqrt(1024)
    out = softmax(S, axis=-1) @ V

Math restructure (identical result, minimal device FLOPs):
    At = Wq.T @ Wk                       (host weight folding, [c, b])
    Pt[b, q]  = sum_c At[c, b] xt[c, q]  [1024, 512] per-core
    St[k, q]  = sum_b xt[b, k] Pt[b, q]  (scores transposed, streamed)
    E         = exp(St / 32)             (no max-subtract: |St/32| < ~4)
    esum[p,q] = sum_kb E[kb][p, q]       (DVE adds; host finishes denom)
    Ut[c, q]  = sum_k x[k, c] E[k, q]    (PSUM-accumulated chains)
    out[q,dv] = sum_c Ut[c, q] WvT[c, dv]   (unnormalized, bf16)
    host: out / denom[q]

Each core handles 512 query rows against the full key range: slightly
UNDER 1/8 of the reference FLOP count (the host weight folding removes
the G = Wq @ xblk.T stage), 640 matmuls of [128 x 128 x 512].
All operands are bf16 (full PE rate, half the DMA/SBUF of fp32r, and
fast LDWEIGHTS); PSUM accumulation is fp32 throughout.
"""

import sys

sys.path.insert(0, "/opt/trn_rl_repo")

import ml_dtypes
import numpy as np

import concourse.tile as tile
from concourse import bacc, mybir
from concourse.bass_utils import run_bass_kernel_spmd

F32 = mybir.dt.float32
BF16 = mybir.dt.bfloat16
BF16_NP = ml_dtypes.bfloat16

S = 4096          # sequence length
D = 1024          # d_in == d_out
P = 128           # partitions
NCORES = 8
R = S // NCORES   # query rows per core (512)
NF = 512          # moving free-dim chunk (1 psum bank of fp32)
KSC = 512         # key super-chunk (xt DMA granularity)
NSC = S // KSC    # 8 super-chunks
KB = S // P       # 32 key blocks
DC = D // P       # 8 chunks of the model dim
QC = R // P       # 4 query chunks per core
SCALE = 1.0 / np.sqrt(np.float32(D))

EXP = mybir.ActivationFunctionType.Exp


def build_program():
    nc = bacc.Bacc("TRN2", target_bir_lowering=False, debug=False,
                   num_devices=NCORES)

    x_d = nc.dram_tensor("x", [S, D], BF16, kind="ExternalInput").ap()
    xt_d = nc.dram_tensor("xt", [D, S], BF16, kind="ExternalInput").ap()
    at_d = nc.dram_tensor("at", [D, D], BF16, kind="ExternalInput").ap()
    wvt_d = nc.dram_tensor("wvt", [D, D], BF16, kind="ExternalInput").ap()
    xqt_d = nc.dram_tensor("xqt", [D, R], BF16, kind="ExternalInput").ap()
    out_d = nc.dram_tensor("out", [R, D], BF16, kind="ExternalOutput").ap()
    esum_d = nc.dram_tensor("esum", [P, R], F32, kind="ExternalOutput").ap()

    with tile.TileContext(nc) as tc:
        _emit(tc, x_d, xt_d, at_d, wvt_d, xqt_d, out_d, esum_d)

    nc.compile()
    return nc


def _emit(tc, x_d, xt_d, at_d, wvt_d, xqt_d, out_d, esum_d):
    nc = tc.nc
    from contextlib import ExitStack

    with ExitStack() as ctx:
        ps = ctx.enter_context(tc.tile_pool(name="ps", bufs=8, space="PSUM"))
        early = ctx.enter_context(tc.tile_pool(name="early", bufs=1))
        pt_pool = ctx.enter_context(tc.tile_pool(name="pt", bufs=1))
        es_pool = ctx.enter_context(tc.tile_pool(name="es", bufs=1))
        xnat_pool = ctx.enter_context(tc.tile_pool(name="xnat", bufs=KB))
        xts_pool = ctx.enter_context(tc.tile_pool(name="xts", bufs=24))
        e_pool = ctx.enter_context(tc.tile_pool(name="epool", bufs=KB))
        wvt_pool = ctx.enter_context(tc.tile_pool(name="wvt", bufs=1))
        ut_pool = ctx.enter_context(tc.tile_pool(name="ut", bufs=1))
        out_pool = ctx.enter_context(tc.tile_pool(name="outp", bufs=2))

        # ---- PE warm-up: HAM throttles a cold PE to K=4/8 (~425ns/matmul)
        # and flips to 8/8 only after ~4us of continuous activity.  The PE
        # idles until the first At/xqt chunks land (11.2-13us observed), so
        # run 12 junk matmuls (~4.3us, warm by #10) sized to END inside that
        # arrival window: the real stream then starts at full rate with no
        # idle gap to re-cool the throttle.
        wl = early.tile([P, P], BF16, tag="warml")
        wr = early.tile([P, R], BF16, tag="warmr")
        nc.vector.memset(wl, 0.0)
        nc.vector.memset(wr, 0.0)
        warm_ps = ps.tile([P, R], F32, tag="mm")
        NWARM = 4
        for i in range(NWARM):
            nc.tensor.matmul(warm_ps, wl, wr,
                             start=(i == 0), stop=(i == NWARM - 1))

        xts_sc = {}

        def prefetch_xts(sc):
            tiles = []
            for cb in range(DC):
                t = xts_pool.tile([P, KSC], BF16, tag="xts")
                nc.sync.dma_start(
                    out=t,
                    in_=xt_d[cb * P:(cb + 1) * P, sc * KSC:(sc + 1) * KSC])
                tiles.append(t)
            xts_sc[sc] = tiles

        # ---- prologue DMAs: xqt/At chunk pairs in consumption order.  DMA
        # issue costs ~650ns each on the sync queue, and the chunk pairs
        # pipeline against the ca-outer Pt matmuls below.
        xqt_tiles = []
        at_tiles = []
        for ca in range(DC):
            xq = early.tile([P, R], BF16, tag=f"xqt{ca}")
            nc.sync.dma_start(out=xq, in_=xqt_d[ca * P:(ca + 1) * P, :])
            xqt_tiles.append(xq)
            at = early.tile([P, D], BF16, tag=f"at{ca}")
            if ca == 0:
                # split the stream-gating first chunk across two DMA rings:
                # one ring moves only ~90GB/s, so halves land in ~1.5us
                nc.sync.dma_start(out=at[:, :D // 2],
                                  in_=at_d[:P, :D // 2])
                nc.sync.dma_start(out=at[:, D // 2:],
                                  in_=at_d[:P, D // 2:])
            else:
                nc.sync.dma_start(out=at, in_=at_d[ca * P:(ca + 1) * P, :])
            at_tiles.append(at)
        prefetch_xts(0)
        prefetch_xts(1)

        # ---- Phase Pt: Pt[b, q] = sum_c At[c, b] xqt[c, q] ----
        # ca-outer over 8 parallel PSUM accumulators: matmul ca only needs
        # DMA pair ca, so PE starts as soon as the first pair lands.
        pt_sb = pt_pool.tile([P, DC, R], BF16)
        pt_ps = []
        for _cb in range(DC):
            pt_acc = ps.tile([P, R], F32, tag="mm")
            pt_ps.append(pt_acc)
        for ca in range(DC):
            for cb in range(DC):
                nc.tensor.matmul(
                    pt_ps[cb],
                    at_tiles[ca][:, cb * P:(cb + 1) * P],
                    xqt_tiles[ca],
                    start=(ca == 0), stop=(ca == DC - 1),
                )
        for cb in range(DC):
            if cb % 2 == 0:
                nc.vector.tensor_copy(pt_sb[:, cb, :], pt_ps[cb])
            else:
                nc.scalar.copy(pt_sb[:, cb, :], pt_ps[cb])

        # ---- Phase A: scores streamed over 32 key blocks ----
        esum_sb = es_pool.tile([P, R], F32)
        xnat = []
        e_tiles = []
        for kb in range(KB):
            sc, kin = divmod(kb, KSC // P)
            if kin == 0 and sc + 2 < NSC:
                prefetch_xts(sc + 2)
            # spread the phase-B x loads across phase A, after xts
            xn = xnat_pool.tile([P, D], BF16, tag="xnat")
            nc.sync.dma_start(out=xn, in_=x_d[kb * P:(kb + 1) * P, :])
            xnat.append(xn)

            st_ps = ps.tile([P, R], F32, tag="mm")
            xts = xts_sc[sc]
            for cb in range(DC):
                nc.tensor.matmul(
                    st_ps,
                    xts[cb][:, kin * P:(kin + 1) * P],
                    pt_sb[:, cb, :],
                    start=(cb == 0), stop=(cb == DC - 1),
                )
            et = e_pool.tile([P, R], BF16, tag="e")
            nc.scalar.activation(et, st_ps, EXP, scale=float(SCALE))
            e_tiles.append(et)
            if kb == 0:
                nc.vector.tensor_copy(esum_sb, et)
            else:
                nc.vector.tensor_add(esum_sb, esum_sb, et)
            if sc >= 2 and kin == 0:
                xts_sc.pop(sc - 2, None)

        # wvt is first needed by the out phase; load it during phase B.
        wvt_sb = wvt_pool.tile([P, DC, D], BF16, tag="wvt")
        for cw in range(DC):
            nc.sync.dma_start(out=wvt_sb[:, cw, :],
                              in_=wvt_d[cw * P:(cw + 1) * P, :])
        nc.sync.dma_start(out=esum_d, in_=esum_sb)

        # ---- Phase B: Ut[c, q] accumulated in PSUM over all 32 k-blocks ----
        ut_sb = ut_pool.tile([P, DC, R], BF16)
        for cc in range(DC):
            ut_ps = ps.tile([P, R], F32, tag="mm")
            for kb in range(KB):
                nc.tensor.matmul(
                    ut_ps,
                    xnat[kb][:, cc * P:(cc + 1) * P],
                    e_tiles[kb],
                    start=(kb == 0), stop=(kb == KB - 1),
                )
            if cc % 2 == 0:
                nc.vector.tensor_copy(ut_sb[:, cc, :], ut_ps)
            else:
                nc.scalar.copy(ut_sb[:, cc, :], ut_ps)

        # ---- Phase C: out[q, dv] = sum_c Ut[c, q] WvT[c, dv] (unnormalized;
        # the host divides by the softmax denominator) ----
        for cq in range(QC):
            ot = out_pool.tile([P, D], BF16, tag="out")
            for nd in range(D // NF):
                if cq == QC - 1 and nd == D // NF - 1:
                    # Final group: two 256-wide halves with casts and DMA
                    # issues on parallel engine queues, shortening the
                    # post-last-matmul tail.
                    for h in range(2):
                        lo = nd * NF + h * (NF // 2)
                        ps_h = ps.tile([P, NF // 2], F32, tag="mm")
                        for cc in range(DC):
                            nc.tensor.matmul(
                                ps_h,
                                ut_sb[:, cc, cq * P:(cq + 1) * P],
                                wvt_sb[:, cc, lo:lo + NF // 2],
                                start=(cc == 0), stop=(cc == DC - 1),
                            )
                        if h == 0:
                            nc.vector.tensor_copy(ot[:, lo:lo + NF // 2], ps_h)
                            nc.sync.dma_start(
                                out=out_d[cq * P:(cq + 1) * P, lo:lo + NF // 2],
                                in_=ot[:, lo:lo + NF // 2])
                        else:
                            # last half: one cast, then two quarter DMAs on
                            # separate queues so the final transfers (a
                            # single ring moves only ~90GB/s) run in parallel
                            nc.scalar.copy(ot[:, lo:lo + NF // 2], ps_h)
                            q4 = NF // 4
                            nc.scalar.dma_start(
                                out=out_d[cq * P:(cq + 1) * P, lo:lo + q4],
                                in_=ot[:, lo:lo + q4])
                            nc.gpsimd.dma_start(
                                out=out_d[cq * P:(cq + 1) * P,
                                          lo + q4:lo + 2 * q4],
                                in_=ot[:, lo + q4:lo + 2 * q4])
                    continue
                ps_o = ps.tile([P, NF], F32, tag="mm")
                for cc in range(DC):
                    nc.tensor.matmul(
                        ps_o,
                        ut_sb[:, cc, cq * P:(cq + 1) * P],
                        wvt_sb[:, cc, nd * NF:(nd + 1) * NF],
                        start=(cc == 0), stop=(cc == DC - 1),
                    )
                if nd % 2 == 0:
                    nc.vector.tensor_copy(ot[:, nd * NF:(nd + 1) * NF], ps_o)
                else:
                    nc.scalar.copy(ot[:, nd * NF:(nd + 1) * NF], ps_o)
                # per-half DMA so the final transfer after the last cast is
                # only 128 KiB
                nc.sync.dma_start(
                    out=out_d[cq * P:(cq + 1) * P, nd * NF:(nd + 1) * NF],
                    in_=ot[:, nd * NF:(nd + 1) * NF])


_CACHE = {}


def _get_program():
    if "nc" not in _CACHE:
        _CACHE["nc"] = build_program()
    return _CACHE["nc"]


def make_in_maps(x, W_query, W_key, W_value):
    x32 = np.ascontiguousarray(x, dtype=np.float32)
    xb = x32.astype(BF16_NP)
    xtb = np.ascontiguousarray(x32.T).astype(BF16_NP)
    at = np.ascontiguousarray(
        np.asarray(W_query, dtype=np.float32).T
        @ np.asarray(W_key, dtype=np.float32)).astype(BF16_NP)
    wvt = np.ascontiguousarray(
        np.asarray(W_value, dtype=np.float32).T).astype(BF16_NP)
    maps = []
    for i in range(NCORES):
        xqt = np.ascontiguousarray(xtb[:, i * R:(i + 1) * R])
        maps.append({"x": xb, "xt": xtb, "at": at, "wvt": wvt, "xqt": xqt})
    return maps


def gather_output(results):
    """Normalize per-core outputs and concatenate to the full [S, D] f32."""
    outs = []
    for i in range(NCORES):
        unnorm = np.asarray(results[i]["out"]).astype(np.float32)
        denom = np.asarray(results[i]["esum"]).astype(np.float32).sum(axis=0)
        outs.append(unnorm / denom[:, None])
    return np.concatenate(outs, axis=0)


def kernel(x, W_query, W_key, W_value):
    nc = _get_program()
    in_maps = make_in_maps(x, W_query, W_key, W_value)
    res = run_bass_kernel_spmd(nc, in_maps, core_ids=list(range(NCORES)))
    return gather_output(res.results)

